# revision 4
# baseline (speedup 1.0000x reference)
"""Trainium2 Bass kernel for nn_CategoryAlign_Module (pooling / cross Pearson).

Math (see reference):
  for each stream s in {1,2}:
    vec_b[k,c]  = sum_p preds[b,k,p] * feats[b,c,p] / sum_p preds[b,k,p]
    ctx_b[k,c]  = vec_b[k,c] / max(||vec_b[:,c]||_2, 1e-12)      (norm over K)
    ctx[k,c]    = mean_b ctx_b[k,c]
  out = pearson(ctx1, ctx2)   (center+normalize rows over C, then ctx1 @ ctx2^T)

Distribution: data-parallel over the batch dim, one batch element per
NeuronCore (B=8, 8 cores).  Each core computes its local normalized
contexts, the tiny [19,257] payload is AllReduce-summed across the 8
cores (Pearson is invariant to the 1/B scale, so the mean's division is
skipped), and every core redundantly computes the replicated [19,19]
correlation.

Per-core pipeline (bf16 compute / fp32 accumulate):
  - both preds and feats are host-relayouted and host-cast to bf16, so
    the device only moves half the bytes and never transposes:
      preds -> [128, 128*19]  (chunk h: P^T[h*128:(h+1)*128, :19])
      feats -> [128, 128*257] (chunk h: [w, c] block with a fused ones
               column, so psum[:, 256] accumulates the mask sums)
  - bulk DMA is split across the two HWDGE queues (sync + scalar
    engines), 16-chunk segments, so both queues stream concurrently
  - 128 accumulating matmuls per stream produce [19, 257] in PSUM
  - stream 0's AllReduce launches at the halfway point and hides under
    stream 1's DMA; only stream 1's AllReduce + the tiny Pearson
    epilogue are exposed at the tail
"""

import sys

sys.path.insert(0, "/opt/trn_rl_repo")

import numpy as np

import concourse.bass as bass  # noqa: F401  (import order matters)
import concourse.bacc as bacc
import concourse.tile as tile
import concourse.mybir as mybir
from concourse import bass_utils, bass2jax  # noqa: F401

B, K, C, H, W = 8, 19, 256, 128, 128
P = H * W            # 16384 spatial positions
NCHUNK = P // 128    # 128 contraction chunks
CCW = C + 1          # channels + fused ones column (mask sums)
SEGC = 16            # chunks per DMA segment
NSEG = NCHUNK // SEGC
N_CORES = 8

F32 = mybir.dt.float32
BF16 = mybir.dt.bfloat16


def build_body(nc, tc, pret_d, ftr_d, identf_d, out_d, n_cores):
    """Emit the per-core program.

    pret_d: 2 DRAM APs [128, NCHUNK*K] bf16 (preds, spatial-major relayout)
    ftr_d:  2 DRAM APs [128, NCHUNK*CCW] bf16 (feats, spatial-major
            relayout + ones column)
    identf_d: [K, K] fp32 identity (for the tiny Pearson transposes)
    out_d:  [K, K] fp32 output
    """
    add = mybir.AluOpType.add
    mult = mybir.AluOpType.mult

    with tc.tile_pool(name="persist", bufs=1) as PP, \
         tc.tile_pool(name="ft", bufs=2 * NSEG) as FTP, \
         tc.tile_pool(name="acc", bufs=1, space="PSUM") as PA, \
         tc.tile_pool(name="tail", bufs=1, space="PSUM") as TLP, \
         tc.tile_pool(name="dram", bufs=1, space="DRAM") as DP:

        # --- constants (identity over SWDGE: also warms up the Q7 rings
        # long before the collectives need them) ---
        id_f = PP.tile([K, K], F32, name="id_f")
        nc.gpsimd.dma_start(id_f[:], identf_d[:])
        ones19 = PP.tile([K, 1], F32, name="ones19")
        nc.vector.memset(ones19[:], 1.0)
        onesrow = PP.tile([1, K], F32, name="onesrow")
        nc.vector.memset(onesrow[:], 1.0)

        # --- bulk DMA: both streams across THREE queues (2 HWDGE + 1
        # SWDGE), interleaved so stream 0 lands in the first half of the
        # DMA window.  3 queues saturate HBM and keep per-engine issue
        # counts near the 4-deep queue limit (no long issue stalls). ---
        qs = [nc.sync, nc.scalar, nc.gpsimd]
        PT = []
        for s in (0, 1):
            pt = PP.tile([128, NCHUNK * K], BF16, name=f"PT{s}")
            qs[s].dma_start(pt[:], pret_d[s][:])
            PT.append(pt)
        fseg = [[], []]
        for s in (0, 1):
            for g in range(NSEG):
                t_ = FTP.tile([128, SEGC * CCW], BF16, name="ftseg")
                qs[(s * NSEG + g) % 3].dma_start(
                    t_[:], ftr_d[s][:, g * SEGC * CCW:(g + 1) * SEGC * CCW])
                fseg[s].append(t_)

        # --- warmups, all hidden under the bulk DMA window: a dummy
        # AllReduce absorbs the collective stack's setup cost and aligns
        # the cores; dummy activations preload the scalar engine's
        # Square/Sqrt/Copy tables so the stream epilogues don't stall on
        # ACT_TABLE_LOAD. ---
        dw_in = DP.tile([1, 1], F32, name="dw_in")
        dw_out = DP.tile([1, 1], F32, name="dw_out")
        nc.gpsimd.dma_start(dw_in[:], ones19[0:1, 0:1])
        prev_cc = nc.gpsimd.collective_compute(
            "AllReduce", add,
            replica_groups=[list(range(n_cores))],
            ins=[dw_in.opt()], outs=[dw_out.opt()])
        wa = PP.tile([1, 1], F32, name="wa")
        wb = PP.tile([1, 1], F32, name="wb")
        nc.scalar.square(wa[:], ones19[0:1, 0:1])
        nc.scalar.sqrt(wb[:], ones19[0:1, 0:1])
        nc.scalar.activation(wa[:], ones19[0:1, 0:1],
                             mybir.ActivationFunctionType.Copy,
                             scale=0.5, accum_out=wb[:])

        # --- per-stream accumulators ---
        psum_vec = [PA.tile([K, CCW], F32, name=f"pvec{s}") for s in (0, 1)]

        csum = []
        nT = []
        for s in (0, 1):
            # ---- contraction: 128 accumulating matmuls ----
            for g in range(NSEG):
                for t in range(SEGC):
                    i = g * SEGC + t
                    nc.tensor.matmul(
                        psum_vec[s][:],
                        lhsT=PT[s][:, i * K:(i + 1) * K],
                        rhs=fseg[s][g][:, t * CCW:(t + 1) * CCW],
                        start=(i == 0), stop=(i == NCHUNK - 1))

            # ---- stream epilogue (stream 0's overlaps stream 1) ----
            # vec = psum[:, :C] / masksum;  ctx = vec / ||vec||_col
            recip = PP.tile([K, 1], F32, name=f"recip{s}")
            nc.vector.reciprocal(recip[:], psum_vec[s][:, C:C + 1])
            rsq = PP.tile([K, 1], F32, name=f"rsq{s}")
            nc.vector.tensor_mul(rsq[:], recip[:], recip[:])
            sq = PP.tile([K, C], F32, name=f"sq{s}")
            nc.scalar.square(sq[:], psum_vec[s][:, 0:C])
            vec_sb = PP.tile([K, C], F32, name=f"vec_sb{s}")
            nc.vector.tensor_scalar_mul(vec_sb[:], psum_vec[s][:, 0:C],
                                        recip[:])
            # col sums over K of (psum^2 * recip^2) = ||vec||^2
            pn = TLP.tile([1, C], F32, name=f"pn{s}", tag="tlp")
            nc.tensor.matmul(pn[:], lhsT=rsq[:], rhs=sq[:],
                             start=True, stop=True)
            # reference clamps the norm at 1e-12; the norm here is
            # O(1e-2) for non-degenerate input, so the clamp is a no-op.
            nsb = PP.tile([1, C], F32, name=f"nsb{s}")
            nc.scalar.sqrt(nsb[:], pn[:])
            rn = PP.tile([1, C], F32, name=f"rn{s}")
            nc.vector.reciprocal(rn[:], nsb[:])
            # broadcast 1/norm to the K partitions (rank-1 matmul)
            bc = TLP.tile([K, C], F32, name=f"bc{s}", tag="tlp")
            nc.tensor.matmul(bc[:], lhsT=onesrow[:], rhs=rn[:],
                             start=True, stop=True)
            cc_in = PP.tile([K, CCW], F32, name=f"cc_in{s}")
            nc.vector.tensor_mul(cc_in[:, 0:C], vec_sb[:], bc[:])
            # ship the per-core row-mean in the payload (mean over B and
            # mean over C commute)
            xdum = PP.tile([K, C], F32, name=f"xdum{s}")
            nc.scalar.activation(xdum[:], cc_in[:, 0:C],
                                 mybir.ActivationFunctionType.Copy,
                                 scale=1.0 / C,
                                 accum_out=cc_in[:, C:C + 1])

            # ---- AllReduce of the tiny [19,257] payload ----
            b_in = DP.tile([K, CCW], F32, name=f"b_in{s}")
            b_out = DP.tile([K, CCW], F32, name=f"b_out{s}")
            nc.gpsimd.dma_start(b_in[:], cc_in[:])
            cc = nc.gpsimd.collective_compute(
                "AllReduce", add,
                replica_groups=[list(range(n_cores))],
                ins=[b_in.opt()], outs=[b_out.opt()])
            bass._add_dep_helper(
                cc.ins, prev_cc.ins, sync=False,
                reason="collectives in stream order")
            prev_cc = cc
            cs = PP.tile([K, CCW], F32, name=f"csum{s}")
            nc.gpsimd.dma_start(cs[:], b_out[:])
            csum.append(cs)

            # ---- side-s Pearson prep (side 0 runs during stream 1;
            # only side 1 trails the last collective) ----
            X = cs[:, 0:C]
            ms = cs[:, C:C + 1]
            xc = PP.tile([K, C], F32, name=f"xc{s}")
            nc.vector.tensor_scalar_sub(xc[:], X, ms)
            xsq = PP.tile([K, C], F32, name=f"xsq{s}")
            ss = PP.tile([K, 1], F32, name=f"ss{s}")
            nc.scalar.activation(xsq[:], xc[:],
                                 mybir.ActivationFunctionType.Square,
                                 accum_out=ss[:])
            sd = PP.tile([K, 1], F32, name=f"sd{s}")
            nc.scalar.sqrt(sd[:], ss[:])
            ri = PP.tile([K, 1], F32, name=f"ri{s}")
            nc.vector.reciprocal(ri[:], sd[:])
            xn = PP.tile([K, C], F32, name=f"xn{s}")
            nc.vector.tensor_scalar(xn[:], X, ms, ri[:],
                                    op0=mybir.AluOpType.subtract,
                                    op1=mult)
            # transpose [K, C] -> [C, K] in two 128-wide blocks
            tps = TLP.tile([128, 2 * K], F32, name=f"tps{s}", tag="tlp")
            for h in (0, 1):
                nc.tensor.matmul(
                    tps[:, h * K:(h + 1) * K],
                    lhsT=xn[:, h * 128:(h + 1) * 128],
                    rhs=id_f[:],
                    is_transpose=True,
                    start=(h == 0), stop=(h == 1))
            nTs = PP.tile([128, 2 * K], F32, name=f"nT{s}")
            nc.vector.tensor_copy(nTs[:], tps[:])
            nT.append(nTs)

        # ---- final correlation ----
        po = TLP.tile([K, K], F32, name="po", tag="tlp")
        for h in (0, 1):
            nc.tensor.matmul(po[:],
                             lhsT=nT[0][:, h * K:(h + 1) * K],
                             rhs=nT[1][:, h * K:(h + 1) * K],
                             start=(h == 0), stop=(h == 1))
        osb = PP.tile([K, K], F32, name="osb")
        nc.vector.tensor_copy(osb[:], po[:])
        nc.sync.dma_start(out_d[:], osb[:])


def build(n_cores=N_CORES):
    nc = bacc.Bacc("TRN2", target_bir_lowering=False, debug=False,
                   enable_asserts=False, num_devices=n_cores)
    pret_d = [nc.dram_tensor(f"pret{s}", [128, NCHUNK * K], BF16,
                             kind="ExternalInput").ap() for s in (1, 2)]
    ftr_d = [nc.dram_tensor(f"ftr{s}", [128, NCHUNK * CCW], BF16,
                            kind="ExternalInput").ap() for s in (1, 2)]
    identf_d = nc.dram_tensor("identf", [K, K], F32, kind="ExternalInput").ap()
    out_d = nc.dram_tensor("out", [K, K], F32, kind="ExternalOutput").ap()
    with tile.TileContext(nc) as tc:
        build_body(nc, tc, pret_d, ftr_d, identf_d, out_d, n_cores)
    nc.compile()
    return nc


_NC_CACHE = {}


def _get_nc():
    if "nc" not in _NC_CACHE:
        _NC_CACHE["nc"] = build(N_CORES)
    return _NC_CACHE["nc"]


class Runner:
    """Executes the compiled Bass program on the first `n_cores` jax
    devices via shard_map, with inputs pre-staged on the devices (the
    analog of the native path's input pre-load in run_neff) so all
    cores start the NEFF near-simultaneously."""

    def __init__(self, nc, n_cores):
        import jax
        from jax.experimental.shard_map import shard_map
        from jax.sharding import Mesh, PartitionSpec, NamedSharding

        bass2jax.install_neuronx_cc_hook()
        self.jax = jax
        self.nc = nc
        self.n_cores = n_cores
        assert nc.dbg_addr is None
        partition_name = (nc.partition_id_tensor.name
                          if nc.partition_id_tensor else None)
        in_names, out_names, out_avals = [], [], []
        for alloc in nc.m.functions[0].allocations:
            if not isinstance(alloc, mybir.MemoryLocationSet):
                continue
            name = alloc.memorylocations[0].name
            if alloc.kind == "ExternalInput":
                if name != partition_name:
                    in_names.append(name)
            elif alloc.kind == "ExternalOutput":
                shape = tuple(alloc.tensor_shape)
                dtype = mybir.dt.np(alloc.dtype)
                out_names.append(name)
                out_avals.append(jax.core.ShapedArray(shape, dtype))
        self.param_names = list(in_names)
        n_params = len(in_names)
        full_in_names = list(in_names) + list(out_names)
        if partition_name is not None:
            full_in_names.append(partition_name)
        full_in_names = tuple(full_in_names)
        donate = tuple(range(n_params, n_params + len(out_names)))
        self.out_names = out_names
        self.out_avals = out_avals

        def _body(*args):
            operands = list(args)
            if partition_name is not None:
                operands.append(bass2jax.partition_id_tensor())
            outs = bass2jax._bass_exec_p.bind(
                *operands,
                out_avals=tuple(out_avals),
                in_names=full_in_names,
                out_names=tuple(out_names),
                lowering_input_output_aliases=(),
                sim_require_finite=True,
                sim_require_nnan=True,
                nc=nc,
            )
            return tuple(outs)

        devices = jax.devices()[:n_cores]
        assert len(devices) == n_cores
        self.mesh = Mesh(np.asarray(devices), ("core",))
        in_specs = (PartitionSpec("core"),) * (n_params + len(out_names))
        out_specs = (PartitionSpec("core"),) * len(out_names)
        self.fn = jax.jit(
            shard_map(_body, mesh=self.mesh, in_specs=in_specs,
                      out_specs=out_specs, check_rep=False),
            donate_argnums=donate, keep_unused=True)
        self.sharding = NamedSharding(self.mesh, PartitionSpec("core"))

    def put(self, in_maps):
        concat = [
            np.concatenate([np.asarray(in_maps[c][n])
                            for c in range(self.n_cores)], axis=0)
            for n in self.param_names
        ]
        arrs = [self.jax.device_put(a, self.sharding) for a in concat]
        self.jax.block_until_ready(arrs)
        return arrs

    def zeros(self):
        zs = [self.jax.device_put(
            np.zeros((self.n_cores * a.shape[0], *a.shape[1:]), a.dtype),
            self.sharding) for a in self.out_avals]
        self.jax.block_until_ready(zs)
        return zs

    def exec(self, dev_in):
        outs = self.fn(*dev_in, *self.zeros())
        self.jax.block_until_ready(outs)
        return {
            name: np.asarray(outs[i]).reshape(
                self.n_cores, *self.out_avals[i].shape)
            for i, name in enumerate(self.out_names)
        }


def _get_runner():
    if "runner" not in _NC_CACHE:
        _NC_CACHE["runner"] = Runner(_get_nc(), N_CORES)
    return _NC_CACHE["runner"]


def make_in_maps(preds1, feats1, preds2, feats2):
    import ml_dtypes
    bf16 = ml_dtypes.bfloat16
    identf = np.eye(K, dtype=np.float32)
    per_stream = {}
    for s, (preds, feats) in enumerate(
            ((preds1, feats1), (preds2, feats2)), start=1):
        # preds [B,K,H,W] -> [B, W(v), H(u), K] -> [B, 128, 128*19]:
        # chunk u's columns are P^T[u*128:(u+1)*128, :19] with the
        # spatial index on partitions
        pr = np.ascontiguousarray(
            preds.astype(bf16).transpose(0, 3, 2, 1)
        ).reshape(B, 128, NCHUNK * K)
        # feats [B,C,H,W] -> [B, W, H, C (+ ones)] -> [B, 128, 128*257]:
        # chunk u is the [w, c] block at h=u, matching pret's chunking;
        # the fused ones column makes psum[:, 256] the mask sums
        ft = np.empty((B, W, H, CCW), dtype=bf16)
        ft[..., :C] = feats.astype(bf16).transpose(0, 3, 2, 1)
        ft[..., C] = 1.0
        per_stream[s] = (pr, ft.reshape(B, 128, NCHUNK * CCW))
    in_maps = []
    for b in range(B):
        in_maps.append({
            "pret1": per_stream[1][0][b],
            "pret2": per_stream[2][0][b],
            "ftr1": per_stream[1][1][b],
            "ftr2": per_stream[2][1][b],
            "identf": identf,
        })
    return in_maps


def kernel(preds1, feats1, preds2, feats2):
    runner = _get_runner()
    in_maps = make_in_maps(preds1, feats1, preds2, feats2)
    dev_in = runner.put(in_maps)
    outs = runner.exec(dev_in)
    return np.asarray(outs["out"][0], dtype=np.float32)
